# revision 2
# baseline (speedup 1.0000x reference)
"""NT-Xent (SimCLR) contrastive loss on 8 Trainium2 NeuronCores.

Reference computation (B=4096, D=256, T=0.5):
    z   = concat(l2norm(x_i), l2norm(x_j))        # [8192, 256]
    sim = z @ z.T                                  # [8192, 8192]
    pos = diag(sim, +B) ++ diag(sim, -B)           # [8192]
    denom_r = sum_{j != r} exp(sim_rj / T)
    loss = mean(-pos/T + log(denom))

Sharding: row-parallel. Core c owns 1024 rows of z. Each core receives
xall rotated by -c*1024 rows so the SPMD program is identical on every
core: "my rows" are always rows 0:1024 of its input, the positive pair
of row r is always row r+4096 (rotation by a multiple of 1024 preserves
the r <-> r+4096 pairing mod 8192), and row sums over ALL columns are
invariant to the column permutation induced by the rotation.

Device program per core:
  A) normalize all 8192 rows: 4-row-tile batched gpsimd cast-DMA
     loads (f32->bf16), DVE fused square+rowsum (scalar_tensor_tensor
     accum), ACT rsqrt via exp(-0.5*ln) on one pinned table set, DVE
     scale -> bf16 zhat, batched stores, DMA-xbar-transpose-load
     zT [2 x [128, 8192]] bf16 per 1024-row strip (pipelined).
  B) column-group-outer matmul loop (starts as soon as the first two
     strips are transposed): per (col-group, row-tile) 8 bf16 matmuls
     into PSUM [128,2048] fp32, one ACT Exp(scale=2) with fused
     accum_out giving per-row partial sums of exp(sim/T).
  C) denom = rowsum - e^2 (removes the j==r self term, sim_rr == 1),
     ACT Ln, DVE fused pos-dots, combine -> [128,1] partials per core.

Host: loss = sum(core partials) / 8192.
"""

import numpy as np

P = 128
D = 256
B = 4096
R = 2 * B                 # 8192 rows of z
NCORES = 8
BLK = R // NCORES         # 1024 rows per core
TILES = R // P            # 64 row tiles of the full z
GT = 4                    # row tiles per DMA group
NGRP = TILES // GT        # 16 groups of 512 rows
NBATCH = 4                # rsqrt batches (4 groups each)
BLK_TILES = BLK // P      # 8 row tiles per core block
T_INV = 2.0               # 1 / TEMP
E2 = float(np.exp(T_INV)) # exp(sim_rr / T) with sim_rr == 1
CG = 2048                 # column group width (PSUM tile free dim)
NG = R // CG              # 4 column groups
KCH = D // P              # 2 contraction chunks of 128

_cached = None


def _build():
    import concourse.bacc as bacc
    import concourse.mybir as mybir
    from concourse import tile

    f32 = mybir.dt.float32
    bf16 = mybir.dt.bfloat16
    AF = mybir.ActivationFunctionType
    ALU = mybir.AluOpType

    # Steer every activation to the one table set containing both Exp and
    # Ln, so the kernel performs a single ACT_TABLE_LOAD instead of
    # thrashing between exp_and_others / natural_log (1.3us per reload).
    # Set names and order are preserved (set ids are positional).
    from concourse import hw_specs as _hw

    _orig_gat = _hw.get_activation_tables

    def _gat_patched(arch):
        tabs = _orig_gat(arch)
        for name, fns in tabs.items():
            if name != "natural_log_exp_and_others":
                fns.discard(AF.Exp)
                fns.discard(AF.Ln)
        return tabs

    bacc.get_activation_tables = _gat_patched

    nc = bacc.Bacc(None, target_bir_lowering=False, debug=False)
    xall = nc.dram_tensor("xall", [R, D], f32, kind="ExternalInput")
    out_d = nc.dram_tensor("out", [P, 1], f32, kind="ExternalOutput")

    def _emit(tc):
        with (
            tc.tile_pool(name="xpool", bufs=8) as xpool,
            tc.tile_pool(name="zkeep", bufs=4) as zkpool,
            tc.tile_pool(name="small", bufs=1) as small,
            tc.tile_pool(name="scratch", bufs=1) as scratch,
            tc.tile_pool(name="ztp", bufs=1) as ztp,
            tc.tile_pool(name="escp", bufs=2) as escp,
            tc.tile_pool(name="dramp", bufs=1, space="DRAM") as dramp,
            tc.tile_pool(name="psum", bufs=2, space="PSUM") as psum,
        ):
            # ---------- Phase A: batched bf16 cast-loads + sumsq ----------
            nsq = small.tile([P, TILES], f32, name="nsq")
            u = small.tile([P, TILES], f32, name="u")
            zhat_dram = dramp.tile([R, D], bf16, name="zhat_dram")
            zT = [
                ztp.tile([P, R], bf16, name="zT0", tag="zT0"),
                ztp.tile([P, R], bf16, name="zT1", tag="zT1"),
            ]
            zk = {}

            xgs = []
            for g in range(NGRP):
                xg = xpool.tile([P, GT, D], bf16, name=f"xg{g}", tag="xg",
                                bufs=8)
                src = xall[g * GT * P:(g + 1) * GT * P, :]
                nc.gpsimd.dma_start(xg[:],
                                    src.rearrange("(t p) d -> p t d", p=P))
                for t in range(GT):
                    j = g * GT + t
                    sqs = scratch.tile([P, D], bf16, name="sqs", tag="sqs",
                                       bufs=2)
                    nc.vector.scalar_tensor_tensor(
                        out=sqs[:], in0=xg[:, t, :], scalar=1.0,
                        in1=xg[:, t, :], op0=ALU.mult, op1=ALU.mult,
                        accum_out=nsq[:, j:j + 1])
                xgs.append(xg)

                if (g + 1) % (NGRP // NBATCH) == 0:
                    b = g // (NGRP // NBATCH)
                    W = TILES // NBATCH                      # 16 tiles/batch
                    sl = slice(b * W, (b + 1) * W)
                    # u = exp(-0.5 ln nsq): one ACT table set for Ln+Exp
                    lnn = scratch.tile([P, W], f32, name="lnn", tag="lnn",
                                       bufs=2)
                    nc.scalar.activation(lnn[:], nsq[:, sl], AF.Ln)
                    nc.scalar.activation(u[:, sl], lnn[:], AF.Exp, scale=-0.5)

                    # normalize + cast + store the batch's groups
                    for gg in range(b * (NGRP // NBATCH),
                                    (b + 1) * (NGRP // NBATCH)):
                        keep = gg in (0, 1, 8, 9)
                        if keep:
                            zg = zkpool.tile([P, GT, D], bf16, name=f"zk{gg}",
                                             tag="zk", bufs=4)
                            zk[gg] = zg
                        else:
                            zg = scratch.tile([P, GT, D], bf16, name="zg",
                                              tag="zg", bufs=3)
                        for t in range(GT):
                            j = gg * GT + t
                            nc.vector.tensor_scalar_mul(
                                zg[:, t, :], xgs[gg][:, t, :], u[:, j:j + 1])
                        dst = zhat_dram[gg * GT * P:(gg + 1) * GT * P, :]
                        nc.sync.dma_start(
                            dst.rearrange("(t p) d -> p t d", p=P), zg[:])
                        # transpose-load any completed 1024-row strip
                        if gg % 2 == 1:
                            s = gg // 2
                            rs = slice(s * BLK, (s + 1) * BLK)
                            for k in range(KCH):
                                nc.sync.dma_start_transpose(
                                    zT[k][:, s * BLK:(s + 1) * BLK],
                                    zhat_dram[rs, k * P:(k + 1) * P])

            # ---------- Phase B: sim tiles, exp, fused row-sums ----------
            acc = small.tile([P, BLK_TILES * NG], f32, name="acc")
            for gc in range(NG):
                for m in range(BLK_TILES):
                    ps = psum.tile([P, CG], f32, name="ps", tag="ps", bufs=2)
                    for k in range(KCH):
                        lhs = zT[k][:, m * P:(m + 1) * P]
                        prev = None
                        for s5 in range(CG // 512):
                            c0 = gc * CG + s5 * 512
                            mm = nc.tensor.matmul(
                                ps[:, s5 * 512:(s5 + 1) * 512], lhs,
                                zT[k][:, c0:c0 + 512],
                                start=(k == 0), stop=(k == KCH - 1))
                            if s5 > 0:
                                # identical stationary operand as previous
                                # matmul: skip the redundant LDWEIGHTS
                                # (PE program order keeps weights resident)
                                mm.ins.ldweights = False
                            prev = mm
                    esc = escp.tile([P, CG], bf16, name="esc", tag="esc",
                                    bufs=2)
                    nc.scalar.activation(
                        esc[:], ps[:], AF.Exp, scale=T_INV,
                        accum_out=acc[:, m * NG + gc:m * NG + gc + 1])

            # ---------- Phase C ----------
            dsum = small.tile([P, BLK_TILES], f32, name="dsum")
            nc.vector.tensor_reduce(
                dsum[:], acc[:].rearrange("p (m g) -> p m g", g=NG),
                axis=mybir.AxisListType.X, op=ALU.add)
            dsub = small.tile([P, BLK_TILES], f32, name="dsub")
            nc.vector.tensor_scalar_add(dsub[:], dsum[:], -E2)
            lnd = small.tile([P, BLK_TILES], f32, name="lnd")
            nc.scalar.activation(lnd[:], dsub[:], AF.Ln)

            posd = small.tile([P, BLK_TILES], f32, name="posd")
            for m in range(BLK_TILES):
                zsl = zk[m // GT][:, m % GT, :]
                psl = zk[8 + m // GT][:, m % GT, :]
                pscr = scratch.tile([P, D], f32, name="pscr", tag="pscr",
                                    bufs=2)
                nc.vector.scalar_tensor_tensor(
                    out=pscr[:], in0=zsl, scalar=1.0, in1=psl,
                    op0=ALU.mult, op1=ALU.mult,
                    accum_out=posd[:, m:m + 1])


            l1 = small.tile([P, 1], f32, name="l1")
            nc.vector.tensor_reduce(l1[:], lnd[:], axis=mybir.AxisListType.X,
                                    op=ALU.add)
            p1 = small.tile([P, 1], f32, name="p1")
            nc.vector.tensor_reduce(p1[:], posd[:], axis=mybir.AxisListType.X,
                                    op=ALU.add)
            p2 = small.tile([P, 1], f32, name="p2")
            nc.vector.tensor_scalar_mul(p2[:], p1[:], -T_INV)
            comb = small.tile([P, 1], f32, name="comb")
            nc.vector.tensor_add(comb[:], l1[:], p2[:])
            nc.sync.dma_start(out_d[:, :], comb[:])

    with tile.TileContext(nc) as tc:
        _emit(tc)
    nc.compile()
    return nc


def _get_nc():
    global _cached
    if _cached is None:
        _cached = _build()
    return _cached


def _make_in_maps(x_i, x_j):
    xall = np.concatenate(
        [np.asarray(x_i, dtype=np.float32), np.asarray(x_j, dtype=np.float32)],
        axis=0,
    )
    return [
        {"xall": np.ascontiguousarray(np.roll(xall, -c * BLK, axis=0))}
        for c in range(NCORES)
    ]


def run(x_i, x_j, trace=False, tmpdir=None):
    """Run on the 8 NeuronCores; returns (loss, BassKernelResults)."""
    from concourse import bass_utils

    nc = _get_nc()
    in_maps = _make_in_maps(x_i, x_j)
    res = bass_utils.run_bass_kernel_spmd(
        nc, in_maps, core_ids=list(range(NCORES)), trace=trace, tmpdir=tmpdir,
    )
    total = np.float32(0.0)
    for r in res.results:
        total += np.float32(np.sum(r["out"], dtype=np.float32))
    loss = np.float32(total / np.float32(R))
    return loss, res


def kernel(x_i, x_j):
    loss, _ = run(x_i, x_j, trace=False)
    return loss

